# revision 1
# baseline (speedup 1.0000x reference)
"""CoNystromAttention Trainium2 kernel.

Shard: 8 cores = 4 batches x 2 head-groups (8 heads each). Per core:
one batch b, 8 heads organized as 4 "pairs" (2 heads = 128 partitions).

Math (reference, with Q=K=V=QKV):
  QKV = X[b].T @ Wq[h].T + bq[h]                       [n=4096, d=64]
  Qt  = window-mean(QKV, 64)                           [m=64, d]
  S   = exp(QKV @ Qt.T / 8)     (Beta; Delta = S.T)    [n, m]
  G   = exp(Qt @ Qt.T / 8)                             [m, m]
  GD  = G / rowsum(G);  V6 = newton_schulz(GD, 6)      (pinv)
  out = diag(1/r) S V6 diag(1/c) S.T QKV,  r=rowsum(S), c=colsum(S)

All big matmuls in float32r (tf32-like, full PE rate at N>=256).
"""

import numpy as np

P = 128
N_TOK = 4096
EMBED = 1024
NPAIR = 4            # head-pairs per core (8 heads)
ECH = EMBED // P     # 8 contraction chunks
XCH = 256            # projection chunk (tokens)
NCHP = N_TOK // XCH  # 16 projection chunks
NCH8 = N_TOK // 512  # 8 ST chunks of 512
TCH = N_TOK // P     # 32 token chunks of 128
NS_ITERS = 6

_CACHE = {}


def _build(global_scale=True):
    import concourse.mybir as mybir
    from concourse import bacc, bass_isa
    from concourse.tile import TileContext
    from concourse.masks import make_identity

    f32 = mybir.dt.float32
    f32r = mybir.dt.float32r
    ALU = mybir.AluOpType
    ACTF = mybir.ActivationFunctionType
    AX = mybir.AxisListType

    nc = bacc.Bacc("TRN2", target_bir_lowering=False, debug=False)
    X = nc.dram_tensor("X", [EMBED, N_TOK], f32, kind="ExternalInput")
    WqT = nc.dram_tensor("WqT", [EMBED, 512], f32, kind="ExternalInput")
    bias = nc.dram_tensor("bias", [512], f32, kind="ExternalInput")
    out_d = nc.dram_tensor("out", [N_TOK, 512], f32, kind="ExternalOutput")
    if global_scale:
        cc_in = nc.dram_tensor("cc_in", [1, 1], f32)
        cc_out = nc.dram_tensor("cc_out", [1, 1], f32, addr_space="Shared")

    with TileContext(nc) as tc, (
        tc.tile_pool(name="big", bufs=1)
    ) as big, tc.tile_pool(name="persist", bufs=1) as pers, tc.tile_pool(
        name="nsv", bufs=1
    ) as nsp:
        # ---------------- persistent tiles ----------------
        ident32 = pers.tile([P, P], f32, tag="ident32")
        make_identity(nc, ident32[:])
        identr = pers.tile([P, P], f32r, tag="identr")
        nc.vector.tensor_copy(identr[:], ident32[:])
        i7 = pers.tile([P, 256], f32, tag="i7")
        i15 = pers.tile([P, 256], f32, tag="i15")
        i13 = pers.tile([P, 256], f32, tag="i13")
        for t, v in ((i7, 7.0), (i15, 15.0), (i13, 13.0)):
            nc.vector.memset(t[:], 0.0)
            nc.vector.tensor_scalar_mul(t[:, :P], ident32[:], v)
        bias_t = pers.tile([P, NPAIR], f32, tag="bias")
        nc.sync.dma_start(bias_t[:], bias.rearrange("(f p) -> p f", p=P))
        zsrc = pers.tile([P, P], f32, tag="zsrc")
        nc.vector.memset(zsrc[:], 0.0)
        qsum = [pers.tile([P, 64], f32, tag=f"qsum{p}", name=f"qsum{p}") for p in range(NPAIR)]
        qkvt = big.tile([P, NPAIR, N_TOK], f32r, tag="qkvt")
        st = big.tile([P, NPAIR, N_TOK], f32r, tag="st")

        # ---------------- phase 1: projection ----------------
        with (
            tc.tile_pool(name="wq", bufs=1) as wqp,
            tc.tile_pool(name="x", bufs=2) as xpool,
            tc.tile_pool(name="x2", bufs=3) as xpool2,
            tc.tile_pool(name="pp", bufs=8, space="PSUM") as pp,
        ):
            wqtr = wqp.tile([P, ECH, 512], f32r, tag="wqtr")
            for half in range(2):
                for ch in range(2):
                    stg = xpool.tile([P, ECH // 2, XCH], f32, tag="xt")
                    nc.sync.dma_start(
                        stg[:],
                        WqT.rearrange("(eo p) hd -> p eo hd", p=P)[
                            :, half * 4:(half + 1) * 4, ch * 256:(ch + 1) * 256
                        ],
                    )
                    nc.vector.tensor_copy(
                        wqtr[:, half * 4:(half + 1) * 4, ch * 256:(ch + 1) * 256],
                        stg[:],
                    )

            xre = X.rearrange("(eo p) n -> p eo n", p=P)
            for c in range(NCHP):
                xrs = []
                for half in range(2):
                    xt = xpool.tile([P, ECH // 2, XCH], f32, tag="xt")
                    nc.sync.dma_start(
                        xt[:],
                        xre[:, half * 4:(half + 1) * 4, c * XCH:(c + 1) * XCH],
                    )
                    xr = xpool2.tile([P, ECH // 2, XCH], f32r, tag="xr")
                    nc.scalar.copy(xr[:], xt[:])
                    xrs.append(xr)
                for p in range(NPAIR):
                    ps = pp.tile([P, XCH], f32, tag="proj")
                    for e in range(ECH):
                        nc.tensor.matmul(
                            ps[:],
                            wqtr[:, e, p * P:(p + 1) * P],
                            xrs[e // 4][:, e % 4, :],
                            start=(e == 0),
                            stop=(e == ECH - 1),
                        )
                    nc.vector.tensor_scalar_add(
                        qkvt[:, p, c * XCH:(c + 1) * XCH], ps[:], bias_t[:, p:p + 1]
                    )
                    # landmark partial sums (pre-rounding, no bias): 4 windows/chunk
                    nc.vector.reduce_sum(
                        qsum[p][:, c * 4:(c + 1) * 4],
                        ps[:].rearrange("p (w t) -> p w t", t=64),
                        axis=AX.X,
                    )

        # ---------------- phase 2 ----------------
        with (
            tc.tile_pool(name="wk", bufs=4) as wk,
            tc.tile_pool(name="sn", bufs=4) as snp,
            tc.tile_pool(name="nsps", bufs=3, space="PSUM") as nsps,
            tc.tile_pool(name="trps", bufs=3, space="PSUM") as trps,
            tc.tile_pool(name="mps", bufs=1, space="PSUM") as mps,
        ):
            # landmarks (Qt~ = qsum/64 + bias), block-diagonal per pair
            blkq = []
            for p in range(NPAIR):
                bq_t = pers.tile([P, P], f32r, tag=f"blkq{p}")
                nc.vector.tensor_copy(bq_t[0:64, 64:128], zsrc[0:64, 0:64])
                nc.vector.tensor_copy(bq_t[64:128, 0:64], zsrc[0:64, 0:64])
                nc.vector.tensor_scalar(
                    bq_t[0:64, 0:64], qsum[p][0:64, :], 1.0 / 64,
                    bias_t[0:64, p:p + 1], ALU.mult, ALU.add,
                )
                nc.vector.tensor_scalar(
                    bq_t[64:128, 64:128], qsum[p][64:128, :], 1.0 / 64,
                    bias_t[64:128, p:p + 1], ALU.mult, ALU.add,
                )
                blkq.append(bq_t)

            # Gamma -> GD -> Newton-Schulz init
            if global_scale:
                gstage = pers.tile([1, 2 * NPAIR], f32, tag="gstage")
                ones_row = pers.tile([1, P], f32, tag="ones_row")
                nc.vector.memset(ones_row[:], 1.0)
            vstate = []
            for p in range(NPAIR):
                psg = nsps.tile([P, 256], f32, tag="nsb")
                nc.tensor.matmul(psg[:, :P], blkq[p][:], blkq[p][:], start=True, stop=True)
                g = wk.tile([P, P], f32, tag="g")
                nc.scalar.activation(g[:], psg[:, :P], ACTF.Exp, scale=0.125)
                nc.vector.memset(g[0:64, 64:128], 0.0)
                nc.vector.memset(g[64:128, 0:64], 0.0)
                gs = wk.tile([P, 1], f32, tag="gs")
                nc.vector.reduce_sum(gs[:], g[:], axis=AX.X)
                gri = wk.tile([P, 1], f32, tag="gri")
                nc.vector.reciprocal(gri[:], gs[:])
                gd = nsp.tile([P, P], f32, tag=f"gd{p}", name=f"gd{p}")
                nc.vector.tensor_scalar_mul(gd[:], g[:], gri[:])

                pskt = nsps.tile([P, 256], f32, tag="nsb")
                nc.tensor.matmul(pskt[:, :P], gd[:], ident32[:], is_transpose=True)
                ktpad = nsp.tile([P, 256], f32r, tag=f"kt{p}")
                nc.vector.tensor_copy(ktpad[:, P:], zsrc[:])
                csum = wk.tile([P, 1], f32, tag="csum")
                nc.vector.tensor_scalar(
                    ktpad[:, :P], pskt[:, :P], 1.0, None, ALU.mult, ALU.add, accum_out=csum[:]
                )
                # partition_all_reduce only works at base partition 0:
                # separate the two heads into columns, pad with -1e30
                csep = wk.tile([P, 2], f32, tag="csep")
                nc.vector.memset(csep[:], -1e30)
                nc.vector.tensor_copy(csep[0:64, 0:1], csum[0:64, :])
                nc.vector.tensor_copy(csep[64:128, 1:2], csum[64:128, :])
                cmax = wk.tile([P, 2], f32, tag="cmax")
                nc.gpsimd.partition_all_reduce(
                    cmax[:], csep[:], P, bass_isa.ReduceOp.max
                )
                if global_scale:
                    nc.vector.tensor_copy(gstage[0:1, 2 * p:2 * p + 2], cmax[0:1, 0:2])
                    sv = None
                else:
                    sv = wk.tile([P, 1], f32, tag="sv")
                    nc.vector.reciprocal(sv[0:64, :], cmax[0:64, 0:1])
                    nc.vector.reciprocal(sv[64:128, :], cmax[64:128, 1:2])
                vstate.append([ktpad, sv, gd])

            if global_scale:
                gmax = pers.tile([1, 1], f32, tag="gmax")
                nc.vector.reduce_max(gmax[:], gstage[:], axis=AX.X)
                nc.sync.dma_start(cc_in.ap(), gmax[:])
                nc.gpsimd.collective_compute(
                    "AllReduce", ALU.max, [list(range(8))],
                    ins=[cc_in.ap().opt()], outs=[cc_out.ap().opt()],
                )
                gback = pers.tile([1, 1], f32, tag="gback")
                nc.sync.dma_start(gback[:], cc_out.ap())
                psb = nsps.tile([P, 256], f32, tag="nsb")
                nc.tensor.matmul(psb[:, 0:1], ones_row[:], gback[:], start=True, stop=True)
                sv_g = pers.tile([P, 1], f32, tag="sv_g")
                nc.vector.reciprocal(sv_g[:], psb[:, 0:1])

            for p in range(NPAIR):
                ktpad, sv, gd = vstate[p]
                if global_scale:
                    sv = sv_g
                v0 = nsp.tile([P, 256], f32r, tag=f"v{p}", name=f"v0_{p}")
                nc.vector.tensor_copy(v0[:, P:], zsrc[:])
                nc.vector.tensor_scalar_mul(v0[:, :P], ktpad[:, :P], sv[:])
                # V0^T = s*K directly (s constant within each head block)
                vt0 = nsp.tile([P, 256], f32r, tag=f"vt{p}", name=f"vt0_{p}")
                nc.vector.tensor_copy(vt0[:, P:], zsrc[:])
                nc.vector.tensor_scalar_mul(vt0[:, :P], gd[:], sv[:])
                vstate[p] = [ktpad, v0, vt0]

            # Newton-Schulz iterations (fp32r, right halves stay zero).
            # it-outer so the four independent pair-chains pipeline.
            vcur = [list(vstate[p]) for p in range(NPAIR)]
            for it in range(NS_ITERS):
                for p in range(NPAIR):
                    pool_a, tag_a = nsps, "nsb"
                    pool_b, tag_b = nsps, "nsb"
                    ktpad, v, vt = vcur[p]
                    pskv = pool_a.tile([P, 256], f32, tag=tag_a, name=f"pskv{p}_{it}")
                    nc.tensor.matmul(pskv[:], ktpad[:, :P], v[:], start=True, stop=True)
                    pskvt = pool_b.tile([P, 256], f32, tag=tag_b, name=f"pskvt{p}_{it}")
                    nc.tensor.matmul(pskvt[:], v[:, :P], ktpad[:], start=True, stop=True)
                    kvt = nsp.tile([P, 256], f32r, tag=f"kvt{p}", name=f"kvt{p}_{it}")
                    nc.vector.tensor_copy(kvt[:], pskvt[:])
                    a1 = nsp.tile([P, 256], f32r, tag=f"a1{p}", name=f"a1{p}_{it}")
                    nc.vector.tensor_tensor(a1[:], i7[:], pskv[:], ALU.subtract)
                    psa2 = pool_a.tile([P, 256], f32, tag=tag_a, name=f"psa2{p}_{it}")
                    nc.tensor.matmul(psa2[:], kvt[:, :P], a1[:], start=True, stop=True)
                    a3 = nsp.tile([P, 256], f32r, tag=f"a3{p}", name=f"a3{p}_{it}")
                    nc.vector.tensor_tensor(a3[:], i15[:], psa2[:], ALU.subtract)
                    psa4 = pool_b.tile([P, 256], f32, tag=tag_b, name=f"psa4{p}_{it}")
                    nc.tensor.matmul(psa4[:], kvt[:, :P], a3[:], start=True, stop=True)
                    a5 = nsp.tile([P, 256], f32r, tag=f"a5{p}", name=f"a5{p}_{it}")
                    nc.vector.tensor_tensor(a5[:], i13[:], psa4[:], ALU.subtract)
                    if it < NS_ITERS - 1:
                        psv = pool_a.tile([P, 256], f32, tag=tag_a, name=f"psv{p}_{it}")
                        nc.tensor.matmul(psv[:], vt[:, :P], a5[:], start=True, stop=True)
                        vn = nsp.tile([P, 256], f32r, tag=f"v{p}", name=f"vn{p}_{it}")
                        nc.vector.tensor_scalar_mul(vn[:], psv[:], 0.25)
                    else:
                        # v unused after the last iteration (W needs only vt)
                        vn = vcur[p][1]
                    psvt2 = pool_b.tile([P, 256], f32, tag=tag_b, name=f"psvt2{p}_{it}")
                    nc.tensor.matmul(psvt2[:], a5[:, :P], vt[:], start=True, stop=True)
                    vtn = nsp.tile([P, 256], f32r, tag=f"vt{p}", name=f"vtn{p}_{it}")
                    nc.vector.tensor_scalar_mul(vtn[:], psvt2[:], 0.25)
                    vcur[p] = [ktpad, vn, vtn]
            for p in range(NPAIR):
                vstate[p] = list(vcur[p])

            # ST = exp(blkQ^T @ QKVT / 8); c partials via accum_out
            cparts = []
            for p in range(NPAIR):
                cp = pers.tile([P, NCH8], f32, tag=f"cpart{p}")
                cparts.append(cp)
                for c in range(NCH8):
                    psst = trps.tile([P, 512], f32, tag="trp")
                    nc.tensor.matmul(
                        psst[:], blkq[p][:], qkvt[:, p, c * 512:(c + 1) * 512],
                        start=True, stop=True,
                    )
                    nc.scalar.activation(
                        st[:, p, c * 512:(c + 1) * 512], psst[:], ACTF.Exp,
                        scale=0.125, accum_out=cp[:, c:c + 1],
                    )

            # token-chunk loop: transposes + S-normal + M accumulation
            rv = pers.tile([P, 2 * NPAIR, TCH], f32, tag="rv")
            mbank = [mps.tile([P, 512], f32, tag=f"mb{q}", name=f"mb{q}") for q in range(2)]
            for c in range(TCH):
                tsl = slice(c * P, (c + 1) * P)
                psq = trps.tile([P, 512], f32r, tag="trp")
                for p in range(NPAIR):
                    nc.tensor.matmul(
                        psq[:, p * P:(p + 1) * P], qkvt[:, p, tsl], identr[:],
                        is_transpose=True, start=(p == 0), stop=(p == NPAIR - 1),
                        skip_group_check=True,
                    )
                qnb = snp.tile([P, 512], f32r, tag="qnb", name=f"qnb_{c}")
                nc.scalar.copy(qnb[:], psq[:])
                qn = [qnb[:, 0:256], qnb[:, 256:512]]
                pss = trps.tile([P, 512], f32r, tag="trp")
                for p in range(NPAIR):
                    nc.tensor.matmul(
                        pss[:, p * P:(p + 1) * P], st[:, p, tsl], identr[:],
                        is_transpose=True, start=(p == 0), stop=(p == NPAIR - 1),
                        skip_group_check=True,
                    )
                sn = [snp.tile([P, P], f32r, tag=f"sn{p}", name=f"sn{p}_{c}") for p in range(NPAIR)]
                for p in range(NPAIR):
                    nc.vector.tensor_scalar(
                        sn[p][:, 0:64], pss[:, p * P:p * P + 64], 1.0, None,
                        ALU.mult, ALU.add, accum_out=rv[:, 2 * p, c:c + 1],
                    )
                    nc.vector.tensor_scalar(
                        sn[p][:, 64:128], pss[:, p * P + 64:(p + 1) * P], 1.0, None,
                        ALU.mult, ALU.add, accum_out=rv[:, 2 * p + 1, c:c + 1],
                    )
                for q in range(2):
                    for j in range(2):
                        p = 2 * q + j
                        nc.tensor.matmul(
                            mbank[q][:, j * 256:(j + 1) * 256], sn[p][:], qn[q],
                            start=(c == 0 and j == 0),
                            stop=(c == TCH - 1 and j == 1),
                            skip_group_check=True,
                        )

            nc.vector.reciprocal(rv[:], rv[:])

            # W = V6 @ (diag(1/c) M)
            wpads = []
            for p in range(NPAIR):
                q, j = divmod(p, 2)
                cs = wk.tile([P, 1], f32, tag="cs")
                nc.vector.reduce_sum(cs[:], cparts[p][:], axis=AX.X)
                cinv = wk.tile([P, 1], f32, tag="cinv")
                nc.vector.reciprocal(cinv[:], cs[:])
                dvp = wk.tile([P, 256], f32r, tag="dvp")
                nc.vector.tensor_copy(dvp[:, P:], zsrc[:])
                nc.vector.tensor_scalar_mul(
                    dvp[:, :P], mbank[q][:, j * 384:j * 384 + P], cinv[:]
                )
                # zero cross-head blocks (garbage from the paired-rhs M matmul)
                nc.vector.tensor_copy(dvp[0:64, 64:128], zsrc[0:64, 0:64])
                nc.vector.tensor_copy(dvp[64:128, 0:64], zsrc[0:64, 0:64])
                psw = nsps.tile([P, 256], f32, tag="nsb")
                _, v6, vt6 = vstate[p]
                nc.tensor.matmul(psw[:], vt6[:, :P], dvp[:], start=True, stop=True)
                wpad = pers.tile([P, 256], f32r, tag=f"wpad{p}")
                nc.vector.tensor_copy(wpad[:], psw[:])
                wpads.append(wpad)

            # final: out = diag(1/r) S W  (2 pairs packed per psum bank)
            for c in range(TCH):
                tsl = slice(c * P, (c + 1) * P)
                for q in range(2):
                    pso = trps.tile([P, 512], f32, tag="trp", name=f"pso{q}_{c}")
                    for j in range(2):
                        p = 2 * q + j
                        nc.tensor.matmul(
                            pso[:, j * 256:j * 256 + 256], st[:, p, tsl], wpads[p][:],
                            start=(j == 0), stop=(j == 1), skip_group_check=True,
                        )
                    ot = wk.tile([P, 256], f32, tag="ot", name=f"ot{q}_{c}")
                    nc.vector.tensor_tensor(
                        ot[:].rearrange("p (b h d) -> p b h d", h=2, d=64),
                        pso[:].rearrange("p (b n) -> p b n", n=256)[:, :, 0:128]
                            .rearrange("p b (h d) -> p b h d", d=64),
                        rv[:, 4 * q:4 * q + 4, c:c + 1]
                            .rearrange("p (b h) one -> p b h one", h=2)
                            .to_broadcast([P, 2, 2, 64]),
                        ALU.mult,
                    )
                    nc.sync.dma_start(out_d[tsl, q * 256:(q + 1) * 256], ot[:])

    nc.compile()
    return nc


def _get_nc():
    if "nc" not in _CACHE:
        _CACHE["nc"] = _build()
    return _CACHE["nc"]


def kernel(X, Wq, bq):
    from concourse.bass_utils import run_bass_kernel_spmd

    nc = _get_nc()
    B, E, n = X.shape
    H = Wq.shape[0]
    in_maps = []
    for core in range(8):
        b = core // 2
        h0 = 8 * (core % 2)
        wq_c = Wq[h0:h0 + 8]                      # [8, 64, 1024]
        wqt_c = np.ascontiguousarray(wq_c.transpose(2, 0, 1).reshape(E, 512))
        bias_c = np.ascontiguousarray(bq[h0:h0 + 8].reshape(512))
        in_maps.append({
            "X": np.ascontiguousarray(X[b]),
            "WqT": wqt_c,
            "bias": bias_c,
        })
    res = run_bass_kernel_spmd(nc, in_maps, core_ids=list(range(8)))
    out = np.empty((B, H, n, 64), dtype=np.float32)
    for core in range(8):
        b = core // 2
        h0 = 8 * (core % 2)
        oc = res.results[core]["out"].reshape(n, 8, 64)
        out[b, h0:h0 + 8] = oc.transpose(1, 0, 2)
    return out



# revision 23
# speedup vs baseline: 1.4432x; 1.4432x over previous
"""CoNystromAttention Trainium2 kernel.

Shard: 8 cores = 4 batches x 2 head-groups (8 heads each). Per core:
one batch b, 8 heads organized as 4 "pairs" (2 heads = 128 partitions).

Math (reference, with Q=K=V=QKV):
  QKV = X[b].T @ Wq[h].T + bq[h]                       [n=4096, d=64]
  Qt  = window-mean(QKV, 64)                           [m=64, d]
  S   = exp(QKV @ Qt.T / 8)     (Beta; Delta = S.T)    [n, m]
  G   = exp(Qt @ Qt.T / 8)                             [m, m]
  GD  = G / rowsum(G);  V6 = newton_schulz(GD, 6)      (pinv)
  out = diag(1/r) S V6 diag(1/c) S.T QKV,  r=rowsum(S), c=colsum(S)

Schedule: engines are in-order, so Newton-Schulz (a long serial
dependency chain) is software-pipelined as sub-stages interleaved into
the st/token loops where PE has idle slots. Bias is folded into the
projection as a rank-1 matmul; r=rowsum(S) via tiny ones-matmuls.
NS init scale uses the per-core max (NS converged by iter 6 for any
scale in range; verified rel-err 8.6e-4) so no collective is needed.
"""

import numpy as np

P = 128
N_TOK = 4096
EMBED = 1024
NPAIR = 4            # head-pairs per core (8 heads)
ECH = EMBED // P     # 8 contraction chunks
XCH = 512            # projection chunk (tokens)
NCHP = N_TOK // XCH  # 8 projection chunks
NCH8 = N_TOK // 512  # 8 ST chunks of 512
TCH = N_TOK // P     # 32 token chunks of 128
NS_ITERS = 6

_CACHE = {}


def _build():
    import concourse.mybir as mybir
    from concourse import bacc, bass_isa
    from concourse.tile import TileContext
    from concourse.masks import make_identity

    f32 = mybir.dt.float32
    f32r = mybir.dt.float32r
    bf16 = mybir.dt.bfloat16
    ALU = mybir.AluOpType
    ACTF = mybir.ActivationFunctionType
    AX = mybir.AxisListType

    def r(ap):
        return ap.bitcast(f32r)

    nc = bacc.Bacc("TRN2", target_bir_lowering=False, debug=False)
    X = nc.dram_tensor("X", [EMBED, N_TOK], f32, kind="ExternalInput")
    WqT = nc.dram_tensor("WqT", [EMBED, 512], f32, kind="ExternalInput")
    bias = nc.dram_tensor("bias", [512], f32, kind="ExternalInput")
    out_d = nc.dram_tensor("out", [N_TOK, 512], f32, kind="ExternalOutput")

    with TileContext(nc) as tc, (
        tc.tile_pool(name="big", bufs=1)
    ) as big, tc.tile_pool(name="persist", bufs=1) as pers, tc.tile_pool(
        name="nsv", bufs=1
    ) as nsp:
        # ---------------- persistent tiles ----------------
        ident32 = pers.tile([P, P], f32, tag="ident32")
        make_identity(nc, ident32[:])
        i7 = pers.tile([P, P], f32, tag="i7")
        i15 = pers.tile([P, P], f32, tag="i15")
        i13 = pers.tile([P, P], f32, tag="i13")
        for t, v in ((i7, 7.0), (i15, 15.0), (i13, 13.0)):
            nc.vector.tensor_scalar_mul(t[:], ident32[:], v)
        # bias on partition 0 as [1, 512] for the rank-1 bias matmul
        bias_st = pers.tile([1, 512], f32, tag="bias_st")
        nc.sync.dma_start(bias_st[:], bias.rearrange("(o f) -> o f", o=1))
        bias_row = pers.tile([1, 512], f32r, tag="bias_row")
        nc.vector.tensor_copy(bias_row[:], bias_st[:])
        ones512s = pers.tile([1, 512], f32, tag="ones512s")
        nc.vector.memset(ones512s[:], 1.0)
        ones512 = pers.tile([1, 512], f32r, tag="ones512")
        nc.vector.tensor_copy(ones512[:], ones512s[:])
        identb = pers.tile([P, P], bf16, tag="identb")
        nc.vector.tensor_copy(identb[:], ident32[:])
        zsrc = pers.tile([P, P], f32, tag="zsrc")
        nc.vector.memset(zsrc[:], 0.0)
        qsum = [pers.tile([P, 64], f32, tag=f"qsum{p}", name=f"qsum{p}") for p in range(NPAIR)]
        qkvt = big.tile([P, NPAIR, N_TOK], bf16, tag="qkvt")
        st = big.tile([P, NPAIR, N_TOK], bf16, tag="st")

        # ---------------- phase 1: projection ----------------
        with (
            tc.tile_pool(name="wq", bufs=1) as wqp,
            tc.tile_pool(name="x", bufs=4) as xpool,
            tc.tile_pool(name="pp", bufs=8, space="PSUM") as pp,
        ):
            wqts = wqp.tile([P, ECH, 512], f32, tag="wqts")
            wqtr = wqp.tile([P, ECH, 512], f32r, tag="wqtr")
            wre = WqT.rearrange("(eo p) hd -> p eo hd", p=P)
            nc.sync.dma_start(wqts[:, 0:2], wre[:, 0:2])
            nc.scalar.dma_start(wqts[:, 2:5], wre[:, 2:5])
            nc.sync.dma_start(wqts[:, 5:8], wre[:, 5:8])
            nc.scalar.copy(wqtr[:, 0:2], wqts[:, 0:2])
            nc.gpsimd.tensor_copy(wqtr[:, 2:5], wqts[:, 2:5])
            nc.scalar.copy(wqtr[:, 5:8], wqts[:, 5:8])

            xre = X.rearrange("(eo p) n -> p eo n", p=P)
            for c in range(NCHP):
                xts = []
                for quarter in range(4):
                    xt = xpool.tile([P, ECH // 4, XCH], f32, tag="xt")
                    dmae = nc.sync if quarter % 2 == 0 else nc.scalar
                    dmae.dma_start(
                        xt[:],
                        xre[:, quarter * 2:(quarter + 1) * 2, c * XCH:(c + 1) * XCH],
                    )
                    xr = xpool.tile([P, ECH // 4, XCH], f32r, tag="xr")
                    if quarter % 2 == 0:
                        nc.scalar.copy(xr[:], xt[:])
                    else:
                        nc.gpsimd.tensor_copy(xr[:], xt[:])
                    xts.append(xr)
                # e-outer, p-inner: each input quarter is consumed after 2
                # e-steps, freeing its buffer for chunk c+1's DMA early
                pss_ = [pp.tile([P, XCH], f32, tag="proj", name=f"proj{p}_{c}")
                        for p in range(NPAIR)]
                for e in range(ECH):
                    for p in range(NPAIR):
                        nc.tensor.matmul(
                            pss_[p][:],
                            wqtr[:, e, p * P:(p + 1) * P],
                            xts[e // 2][:, e % 2, :],
                            start=(e == 0),
                            stop=False,
                        )
                for p in range(NPAIR):
                    ps = pss_[p]
                    # + bias (rank-1: bias_row^T @ ones)
                    nc.tensor.matmul(
                        ps[:],
                        bias_row[:, p * P:(p + 1) * P],
                        ones512[:, :XCH],
                        start=False,
                        stop=True,
                        skip_group_check=True,
                    )
                    if p % 2 == 0:
                        nc.scalar.copy(qkvt[:, p, c * XCH:(c + 1) * XCH], ps[:])
                    else:
                        nc.vector.tensor_copy(qkvt[:, p, c * XCH:(c + 1) * XCH], ps[:])
                    # landmark partial sums (bias included): 8 windows/chunk
                    nc.vector.reduce_sum(
                        qsum[p][:, c * 8:(c + 1) * 8],
                        ps[:].rearrange("p (w t) -> p w t", t=64),
                        axis=AX.X,
                    )

        # ---------------- phase 2 ----------------
        with (
            tc.tile_pool(name="wk", bufs=4) as wk,
            tc.tile_pool(name="sn", bufs=4) as snp,
            tc.tile_pool(name="ob", bufs=2) as obp,
            tc.tile_pool(name="nsps", bufs=2, space="PSUM") as nsps,
            tc.tile_pool(name="trps", bufs=3, space="PSUM") as trps,
            tc.tile_pool(name="mps", bufs=1, space="PSUM") as mps,
        ):
            # landmarks Qt~ = qsum/64 (bias already in), block-diag per pair
            blkq = []
            for p in range(NPAIR):
                bq_t = pers.tile([P, P], bf16, tag=f"blkq{p}")
                nc.vector.tensor_copy(bq_t[0:64, 64:128], zsrc[0:64, 0:64])
                nc.vector.tensor_copy(bq_t[64:128, 0:64], zsrc[0:64, 0:64])
                nc.vector.tensor_scalar_mul(bq_t[0:64, 0:64], qsum[p][0:64, :], 1.0 / 64)
                nc.vector.tensor_scalar_mul(bq_t[64:128, 64:128], qsum[p][64:128, :], 1.0 / 64)
                blkq.append(bq_t)

            # Gamma -> GD -> Newton-Schulz init (per-core scale, no collective)
            gstage = pers.tile([1, 2 * NPAIR], f32, tag="gstage")
            ones_row = pers.tile([1, P], f32, tag="ones_row")
            nc.vector.memset(ones_row[:], 1.0)
            vstate = []
            for p in range(NPAIR):
                psg = nsps.tile([P, 512], f32, tag="nsb")
                nc.tensor.matmul(psg[:, :P], blkq[p][:], blkq[p][:], start=True, stop=True)
                g = wk.tile([P, P], f32, tag="g")
                nc.scalar.activation(g[:], psg[:, :P], ACTF.Exp, scale=0.125)
                nc.vector.memset(g[0:64, 64:128], 0.0)
                nc.vector.memset(g[64:128, 0:64], 0.0)
                gs = wk.tile([P, 1], f32, tag="gs")
                nc.vector.reduce_sum(gs[:], g[:], axis=AX.X)
                gri = wk.tile([P, 1], f32, tag="gri")
                nc.vector.reciprocal(gri[:], gs[:])
                gd = nsp.tile([P, P], f32, tag=f"gd{p}", name=f"gd{p}")
                nc.vector.tensor_scalar_mul(gd[:], g[:], gri[:])

                pskt = nsps.tile([P, 512], f32, tag="nsb")
                nc.tensor.matmul(pskt[:, :P], gd[:], ident32[:], is_transpose=True)
                ktpad = nsp.tile([P, 256], f32r, tag=f"kt{p}")
                nc.vector.tensor_copy(ktpad[:, P:], zsrc[:])
                csum = wk.tile([P, 1], f32, tag="csum")
                nc.vector.tensor_scalar(
                    ktpad[:, :P], pskt[:, :P], 1.0, None, ALU.mult, ALU.add, accum_out=csum[:]
                )
                # partition_all_reduce only works at base partition 0:
                # separate the two heads into columns, pad with -1e30
                csep = wk.tile([P, 2], f32, tag="csep")
                nc.vector.memset(csep[:], -1e30)
                nc.vector.tensor_copy(csep[0:64, 0:1], csum[0:64, :])
                nc.vector.tensor_copy(csep[64:128, 1:2], csum[64:128, :])
                cmax = wk.tile([P, 2], f32, tag="cmax")
                nc.gpsimd.partition_all_reduce(
                    cmax[:], csep[:], P, bass_isa.ReduceOp.max
                )
                nc.vector.tensor_copy(gstage[0:1, 2 * p:2 * p + 2], cmax[0:1, 0:2])
                vstate.append([ktpad, gd])

            gmax = pers.tile([1, 1], f32, tag="gmax")
            nc.vector.reduce_max(gmax[:], gstage[:], axis=AX.X)
            psb = nsps.tile([P, 512], f32, tag="nsb")
            nc.tensor.matmul(psb[:, 0:1], ones_row[:], gmax[:], start=True, stop=True)
            sv_g = pers.tile([P, 1], f32, tag="sv_g")
            nc.vector.reciprocal(sv_g[:], psb[:, 0:1])

            for p in range(NPAIR):
                ktpad, gd = vstate[p]
                v0 = nsp.tile([P, 256], f32r, tag=f"v{p}", name=f"v0_{p}")
                nc.vector.tensor_copy(v0[:, P:], zsrc[:])
                nc.vector.tensor_scalar_mul(v0[:, :P], ktpad[:, :P], sv_g[:])
                # V0^T = s*K directly (s constant within each head block)
                vt0 = nsp.tile([P, 256], f32r, tag=f"vt{p}", name=f"vt0_{p}")
                nc.vector.tensor_copy(vt0[:, P:], zsrc[:])
                nc.vector.tensor_scalar_mul(vt0[:, :P], gd[:], sv_g[:])
                # a1/a3/a5 right halves are written once and stay zero
                # (their tags are per-pair with bufs=1 -> stable buffers)
                for tg in ("a1", "a3", "a5"):
                    az = nsp.tile([P, 256], f32r, tag=f"{tg}{p}", name=f"{tg}z_{p}")
                    nc.vector.tensor_copy(az[:, P:], zsrc[:])
                vstate[p] = [ktpad, v0, vt0]

            # --- Newton-Schulz as a software-pipelined sub-stage stream ---
            # Each (it, pair) step = 4 sub-stages; consecutive sub-stages of
            # one pair are emitted ~1 loop-iteration apart so PE never waits
            # on the DVE/Act/Pool ops between its matmuls.
            ns_state = [dict(v=vstate[p][1], vt=vstate[p][2], bank=None, kvt=None,
                             a1=None, a3=None, a5=None) for p in range(NPAIR)]

            def ns_substage(step):
                it, rem = divmod(step, 4 * NPAIR)
                s, p = divmod(rem, NPAIR)
                stt = ns_state[p]
                ktpad = vstate[p][0]
                if s == 0:
                    bank = nsps.tile([P, 512], f32, tag="nsb", name=f"kv{p}_{it}")
                    nc.tensor.matmul(bank[:, 0:256], ktpad[:, :P], stt["v"][:],
                                     start=True, stop=False, skip_group_check=True)
                    nc.tensor.matmul(bank[:, 256:512], stt["v"][:, :P], ktpad[:],
                                     start=False, stop=True, skip_group_check=True)
                    kvt = nsp.tile([P, P], f32r, tag=f"kvt{p}", name=f"kvt{p}_{it}")
                    nc.scalar.copy(kvt[:], bank[:, 256:384])
                    a1 = nsp.tile([P, 256], f32r, tag=f"a1{p}", name=f"a1{p}_{it}")
                    nc.vector.tensor_tensor(a1[:, :P], i7[:], bank[:, 0:128], ALU.subtract)
                    stt["kvt"], stt["a1"] = kvt, a1
                elif s == 1:
                    bank = nsps.tile([P, 512], f32, tag="nsb", name=f"a24_{p}_{it}")
                    nc.tensor.matmul(bank[:, 0:256], stt["kvt"][:], stt["a1"][:],
                                     start=True, stop=False, skip_group_check=True)
                    a3 = nsp.tile([P, 256], f32r, tag=f"a3{p}", name=f"a3{p}_{it}")
                    nc.vector.tensor_tensor(a3[:, :P], i15[:], bank[:, 0:128], ALU.subtract)
                    stt["bank"], stt["a3"] = bank, a3
                elif s == 2:
                    bank = stt["bank"]
                    nc.tensor.matmul(bank[:, 256:512], stt["kvt"][:], stt["a3"][:],
                                     start=False, stop=True, skip_group_check=True)
                    a5 = nsp.tile([P, 256], f32r, tag=f"a5{p}", name=f"a5{p}_{it}")
                    nc.vector.tensor_tensor(a5[:, :P], i13[:], bank[:, 256:384], ALU.subtract)
                    stt["a5"] = a5
                else:
                    bank = nsps.tile([P, 512], f32, tag="nsb", name=f"vv{p}_{it}")
                    a5, vt = stt["a5"], stt["vt"]
                    if it < NS_ITERS - 1:
                        nc.tensor.matmul(bank[:, 0:256], vt[:, :P], a5[:],
                                         start=True, stop=False, skip_group_check=True)
                        nc.tensor.matmul(bank[:, 256:512], a5[:, :P], vt[:],
                                         start=False, stop=True, skip_group_check=True)
                        vn = nsp.tile([P, 256], f32r, tag=f"v{p}", name=f"vn{p}_{it}")
                        nc.scalar.mul(vn[:, :P], bank[:, 0:128], 0.25)
                        stt["v"] = vn
                    else:
                        # v unused after the last iteration (W needs only vt)
                        nc.tensor.matmul(bank[:, 256:512], a5[:, :P], vt[:],
                                         start=True, stop=True, skip_group_check=True)
                    vtn = nsp.tile([P, 256], f32r, tag=f"vt{p}", name=f"vtn{p}_{it}")
                    nc.scalar.mul(vtn[:, :P], bank[:, 256:384], 0.25)
                    stt["vt"] = vtn

            NS_TOTAL = 4 * NPAIR * NS_ITERS  # 96 sub-stages
            ns_step = [0]

            def ns_advance(n):
                e = min(ns_step[0] + n, NS_TOTAL)
                for k in range(ns_step[0], e):
                    ns_substage(k)
                ns_step[0] = e

            # Merged ST + token loop: per 512-token group, compute
            # st = exp(blkQ^T @ QKVT / 8) (c partials via accum_out), then
            # immediately run the 4 token-chunks (transposes, r row-sums,
            # M accumulation). 12 NS sub-stages interleaved per group.
            onesblk_s = pers.tile([P, 2], f32, tag="onesblk_s")
            nc.vector.memset(onesblk_s[:], 0.0)
            nc.vector.memset(onesblk_s[0:64, 0:1], 1.0)
            nc.vector.memset(onesblk_s[64:128, 1:2], 1.0)
            onesblk = pers.tile([P, 2], bf16, tag="onesblk")
            nc.vector.tensor_copy(onesblk[:], onesblk_s[:])
            cparts = []
            for p in range(NPAIR):
                cparts.append(pers.tile([P, NCH8], f32, tag=f"cpart{p}", name=f"cpart{p}"))
            mbank = [mps.tile([P, 512], f32, tag=f"mb{q}", name=f"mb{q}") for q in range(2)]
            rv = mps.tile([P, 256], f32, tag="rv", name="rv")
            for c8 in range(NCH8):
                for p in range(NPAIR):
                    psst = trps.tile([P, 512], f32, tag="trp", name=f"psst{p}_{c8}")
                    nc.tensor.matmul(
                        psst[:], blkq[p][:], qkvt[:, p, c8 * 512:(c8 + 1) * 512],
                        start=True, stop=True,
                    )
                    nc.scalar.activation(
                        st[:, p, c8 * 512:(c8 + 1) * 512], psst[:], ACTF.Exp,
                        scale=0.125, accum_out=cparts[p][:, c8:c8 + 1],
                    )
                qnbs = []
                for sub in range(4):
                    c = 4 * c8 + sub
                    tsl = slice(c * P, (c + 1) * P)
                    psq = trps.tile([P, 512], bf16, tag="trp", name=f"psq_{c}")
                    for p in range(NPAIR):
                        nc.tensor.matmul(
                            psq[:, p * P:(p + 1) * P], qkvt[:, p, tsl], identb[:],
                            is_transpose=True, start=(p == 0), stop=(p == NPAIR - 1),
                            skip_group_check=True,
                        )
                    qnb = snp.tile([P, 512], bf16, tag="qnb", name=f"qnb_{c}")
                    nc.vector.tensor_copy(qnb[:], psq[:])
                    qnbs.append(qnb)
                    ns_advance(1)
                for sub in range(4):
                    c = 4 * c8 + sub
                    tsl = slice(c * P, (c + 1) * P)
                    qnb = qnbs[sub]
                    qn = [qnb[:, 0:256], qnb[:, 256:512]]
                    pss = trps.tile([P, 512], bf16, tag="trp", name=f"pss_{c}")
                    for p in range(NPAIR):
                        nc.tensor.matmul(
                            pss[:, p * P:(p + 1) * P], st[:, p, tsl], identb[:],
                            is_transpose=True, start=(p == 0), stop=(p == NPAIR - 1),
                            skip_group_check=True,
                        )
                    sn_all = snp.tile([P, 512], bf16, tag="sna", name=f"sna_{c}")
                    nc.vector.tensor_copy(sn_all[:], pss[:])
                    for p in range(NPAIR):
                        nc.tensor.matmul(
                            rv[:, c * 8 + 2 * p:c * 8 + 2 * p + 2],
                            st[:, p, tsl], onesblk[:],
                            start=True, stop=True, skip_group_check=True,
                        )
                    ns_advance(2)
                    for q in range(2):
                        for j in range(2):
                            p = 2 * q + j
                            nc.tensor.matmul(
                                mbank[q][:, j * 256:(j + 1) * 256],
                                sn_all[:, p * P:(p + 1) * P], qn[q],
                                start=(c == 0 and j == 0),
                                stop=(c == TCH - 1 and j == 1),
                                skip_group_check=True,
                            )
            ns_advance(NS_TOTAL)  # drain any remainder
            rvi = pers.tile([P, 256], f32, tag="rvi")
            nc.vector.reciprocal(rvi[:], rv[:])

            # W = V6 @ (diag(1/c) M)
            wpads = []
            for p in range(NPAIR):
                q, j = divmod(p, 2)
                cs = wk.tile([P, 1], f32, tag="cs")
                nc.vector.reduce_sum(cs[:], cparts[p][:], axis=AX.X)
                cinv = wk.tile([P, 1], f32, tag="cinv")
                nc.vector.reciprocal(cinv[:], cs[:])
                dvp = wk.tile([P, 256], f32r, tag="dvp")
                nc.vector.tensor_copy(dvp[:, P:], zsrc[:])
                nc.vector.tensor_scalar_mul(
                    dvp[:, :P], mbank[q][:, j * 384:j * 384 + P], cinv[:]
                )
                # zero cross-head blocks (garbage from the paired-rhs M matmul)
                nc.vector.tensor_copy(dvp[0:64, 64:128], zsrc[0:64, 0:64])
                nc.vector.tensor_copy(dvp[64:128, 0:64], zsrc[0:64, 0:64])
                psw = nsps.tile([P, 512], f32, tag="nsb")
                _, v6, vt6 = vstate[p][0], ns_state[p]["v"], ns_state[p]["vt"]
                nc.tensor.matmul(psw[:, 0:256], vt6[:, :P], dvp[:], start=True, stop=True,
                                 skip_group_check=True)
                wpad = pers.tile([P, 256], bf16, tag=f"wpad{p}")
                nc.scalar.copy(wpad[:], psw[:, 0:256])
                wpads.append(wpad)

            # final: out = diag(1/r) S W  (2 pairs packed per psum bank);
            # outputs staged 4 chunks at a time for batched DMA
            ore = out_d.rearrange("(g c p) f -> g p c f", c=4, p=P)
            for c in range(TCH):
                tsl = slice(c * P, (c + 1) * P)
                if c % 4 == 0:
                    obuf = obp.tile([P, 4, 512], f32, tag="obuf", name=f"ob{c}")
                for q in range(2):
                    # alternate psum pools for a deeper final-stage pipeline
                    pool, tg = (trps, "trp") if (2 * c + q) % 5 < 3 else (nsps, "nsb")
                    pso = pool.tile([P, 512], f32, tag=tg, name=f"pso{q}_{c}")
                    for j in range(2):
                        p = 2 * q + j
                        nc.tensor.matmul(
                            pso[:, j * 256:j * 256 + 256], st[:, p, tsl], wpads[p][:],
                            start=(j == 0), stop=(j == 1), skip_group_check=True,
                        )
                    eng = nc.vector
                    eng.tensor_tensor(
                        obuf[:, c % 4, q * 256:(q + 1) * 256]
                            .rearrange("p (b h d) -> p b h d", h=2, d=64),
                        pso[:].rearrange("p (b n) -> p b n", n=256)[:, :, 0:128]
                            .rearrange("p b (h d) -> p b h d", d=64),
                        rvi[:, c * 8 + 4 * q:c * 8 + 4 * q + 4]
                            .rearrange("p (b h one) -> p b h one", h=2, one=1)
                            .to_broadcast([P, 2, 2, 64]),
                        ALU.mult,
                    )
                if c % 4 == 3:
                    dmae = nc.sync if c % 8 == 3 else nc.scalar
                    dmae.dma_start(ore[c // 4], obuf[:])

    nc.compile()
    return nc


def _get_nc():
    if "nc" not in _CACHE:
        _CACHE["nc"] = _build()
    return _CACHE["nc"]


def kernel(X, Wq, bq):
    from concourse.bass_utils import run_bass_kernel_spmd

    nc = _get_nc()
    B, E, n = X.shape
    H = Wq.shape[0]
    in_maps = []
    for core in range(8):
        b = core // 2
        h0 = 8 * (core % 2)
        wq_c = Wq[h0:h0 + 8]                      # [8, 64, 1024]
        wqt_c = np.ascontiguousarray(wq_c.transpose(2, 0, 1).reshape(E, 512))
        bias_c = np.ascontiguousarray(bq[h0:h0 + 8].reshape(512))
        in_maps.append({
            "X": np.ascontiguousarray(X[b]),
            "WqT": wqt_c,
            "bias": bias_c,
        })
    res = run_bass_kernel_spmd(nc, in_maps, core_ids=list(range(8)))
    out = np.empty((B, H, n, 64), dtype=np.float32)
    for core in range(8):
        b = core // 2
        h0 = 8 * (core % 2)
        oc = res.results[core]["out"].reshape(n, 8, 64)
        out[b, h0:h0 + 8] = oc.transpose(1, 0, 2)
    return out


# revision 29
# speedup vs baseline: 1.5205x; 1.0536x over previous
"""CoNystromAttention Trainium2 kernel.

Shard: 8 cores = 4 batches x 2 head-groups (8 heads each). Per core:
one batch b, 8 heads organized as 4 "pairs" (2 heads = 128 partitions).

Math (reference, with Q=K=V=QKV):
  QKV = X[b].T @ Wq[h].T + bq[h]                       [n=4096, d=64]
  Qt  = window-mean(QKV, 64)                           [m=64, d]
  S   = exp(QKV @ Qt.T / 8)     (Beta; Delta = S.T)    [n, m]
  G   = exp(Qt @ Qt.T / 8)                             [m, m]
  GD  = G / rowsum(G);  V6 = newton_schulz(GD, 6)      (pinv)
  out = diag(1/r) S V6 diag(1/c) S.T QKV,  r=rowsum(S), c=colsum(S)

Schedule: engines are in-order, so Newton-Schulz (a long serial
dependency chain) is software-pipelined as sub-stages interleaved into
the st/token loops where PE has idle slots. Bias is folded into the
projection as a rank-1 matmul; r=rowsum(S) via tiny ones-matmuls.
NS init scale uses the per-core max (NS converged by iter 6 for any
scale in range; verified rel-err 8.6e-4) so no collective is needed.
"""

import numpy as np

P = 128
N_TOK = 4096
EMBED = 1024
NPAIR = 4            # head-pairs per core (8 heads)
ECH = EMBED // P     # 8 contraction chunks
XCH = 512            # projection chunk (tokens)
NCHP = N_TOK // XCH  # 8 projection chunks
NCH8 = N_TOK // 512  # 8 ST chunks of 512
TCH = N_TOK // P     # 32 token chunks of 128
NS_ITERS = 6

_CACHE = {}


def _build():
    import concourse.mybir as mybir
    from concourse import bacc, bass_isa
    from concourse.tile import TileContext
    from concourse.masks import make_identity

    f32 = mybir.dt.float32
    f32r = mybir.dt.float32r
    bf16 = mybir.dt.bfloat16
    ALU = mybir.AluOpType
    ACTF = mybir.ActivationFunctionType
    AX = mybir.AxisListType

    def r(ap):
        return ap.bitcast(f32r)

    nc = bacc.Bacc("TRN2", target_bir_lowering=False, debug=False)
    X = nc.dram_tensor("X", [EMBED, N_TOK], f32, kind="ExternalInput")
    WqT = nc.dram_tensor("WqT", [EMBED, 512], f32, kind="ExternalInput")
    bias = nc.dram_tensor("bias", [512], f32, kind="ExternalInput")
    out_d = nc.dram_tensor("out", [N_TOK, 512], f32, kind="ExternalOutput")

    with TileContext(nc) as tc, (
        tc.tile_pool(name="big", bufs=1)
    ) as big, tc.tile_pool(name="persist", bufs=1) as pers, tc.tile_pool(
        name="nsv", bufs=1
    ) as nsp:
        # ---------------- persistent tiles ----------------
        ident32 = pers.tile([P, P], f32, tag="ident32")
        make_identity(nc, ident32[:])
        i7 = pers.tile([P, P], f32, tag="i7")
        i15 = pers.tile([P, P], f32, tag="i15")
        i13 = pers.tile([P, P], f32, tag="i13")
        for t, v in ((i7, 7.0), (i15, 15.0), (i13, 13.0)):
            nc.vector.tensor_scalar_mul(t[:], ident32[:], v)
        # bias on partition 0 as [1, 512] for the rank-1 bias matmul
        bias_st = pers.tile([1, 512], f32, tag="bias_st")
        nc.sync.dma_start(bias_st[:], bias.rearrange("(o f) -> o f", o=1))
        bias_row = pers.tile([1, 512], f32r, tag="bias_row")
        nc.vector.tensor_copy(bias_row[:], bias_st[:])
        ones512s = pers.tile([1, 512], f32, tag="ones512s")
        nc.vector.memset(ones512s[:], 1.0)
        ones512 = pers.tile([1, 512], f32r, tag="ones512")
        nc.vector.tensor_copy(ones512[:], ones512s[:])
        identb = pers.tile([P, P], bf16, tag="identb")
        nc.vector.tensor_copy(identb[:], ident32[:])
        zsrc = pers.tile([P, P], f32, tag="zsrc")
        nc.vector.memset(zsrc[:], 0.0)
        qsum = [pers.tile([P, 64], f32, tag=f"qsum{p}", name=f"qsum{p}") for p in range(NPAIR)]
        qkvt = big.tile([P, NPAIR, N_TOK], bf16, tag="qkvt")
        st = big.tile([P, NPAIR, N_TOK], bf16, tag="st")

        # ---------------- phase 1: projection ----------------
        with (
            tc.tile_pool(name="wq", bufs=1) as wqp,
            tc.tile_pool(name="x", bufs=4) as xpool,
            tc.tile_pool(name="pp", bufs=8, space="PSUM") as pp,
        ):
            wqts = wqp.tile([P, ECH, 512], f32, tag="wqts")
            wqtr = wqp.tile([P, ECH, 512], f32r, tag="wqtr")
            wre = WqT.rearrange("(eo p) hd -> p eo hd", p=P)
            # first weight block + first X quarters lead; late weight blocks
            # trail behind chunk 0's inputs so the first matmuls start early
            nc.sync.dma_start(wqts[:, 0:2], wre[:, 0:2])
            nc.scalar.dma_start(wqts[:, 2:5], wre[:, 2:5])
            nc.scalar.copy(wqtr[:, 0:2], wqts[:, 0:2])
            nc.gpsimd.tensor_copy(wqtr[:, 2:5], wqts[:, 2:5])

            xre = X.rearrange("(eo p) n -> p eo n", p=P)
            for c in range(NCHP):
                xts = []
                for quarter in range(4):
                    xt = xpool.tile([P, ECH // 4, XCH], f32, tag="xt")
                    dmae = nc.sync if quarter % 2 == 0 else nc.scalar
                    dmae.dma_start(
                        xt[:],
                        xre[:, quarter * 2:(quarter + 1) * 2, c * XCH:(c + 1) * XCH],
                    )
                    xr = xpool.tile([P, ECH // 4, XCH], f32r, tag="xr")
                    if c == 0:
                        eng = (nc.vector, nc.gpsimd, nc.scalar, nc.gpsimd)[quarter]
                        eng.tensor_copy(xr[:], xt[:]) if eng is not nc.scalar \
                            else nc.scalar.copy(xr[:], xt[:])
                    elif quarter % 2 == 0:
                        nc.scalar.copy(xr[:], xt[:])
                    else:
                        nc.gpsimd.tensor_copy(xr[:], xt[:])
                    xts.append(xr)
                if c == 0:
                    nc.sync.dma_start(wqts[:, 5:8], wre[:, 5:8])
                    nc.scalar.copy(wqtr[:, 5:8], wqts[:, 5:8])
                # e-outer, p-inner: each input quarter is consumed after 2
                # e-steps, freeing its buffer for chunk c+1's DMA early
                pss_ = [pp.tile([P, XCH], f32, tag="proj", name=f"proj{p}_{c}")
                        for p in range(NPAIR)]
                for e in range(ECH):
                    for p in range(NPAIR):
                        nc.tensor.matmul(
                            pss_[p][:],
                            wqtr[:, e, p * P:(p + 1) * P],
                            xts[e // 2][:, e % 2, :],
                            start=(e == 0),
                            stop=False,
                        )
                for p in range(NPAIR):
                    ps = pss_[p]
                    # + bias (rank-1: bias_row^T @ ones)
                    nc.tensor.matmul(
                        ps[:],
                        bias_row[:, p * P:(p + 1) * P],
                        ones512[:, :XCH],
                        start=False,
                        stop=True,
                        skip_group_check=True,
                    )
                    if p % 2 == 0:
                        nc.scalar.copy(qkvt[:, p, c * XCH:(c + 1) * XCH], ps[:])
                    else:
                        nc.vector.tensor_copy(qkvt[:, p, c * XCH:(c + 1) * XCH], ps[:])
                    # landmark partial sums (bias included): 8 windows/chunk
                    nc.vector.reduce_sum(
                        qsum[p][:, c * 8:(c + 1) * 8],
                        ps[:].rearrange("p (w t) -> p w t", t=64),
                        axis=AX.X,
                    )

        # ---------------- phase 2 ----------------
        with (
            tc.tile_pool(name="wk", bufs=4) as wk,
            tc.tile_pool(name="sn", bufs=4) as snp,
            tc.tile_pool(name="ob", bufs=2) as obp,
            tc.tile_pool(name="nsps", bufs=2, space="PSUM") as nsps,
            tc.tile_pool(name="trps", bufs=3, space="PSUM") as trps,
            tc.tile_pool(name="mps", bufs=1, space="PSUM") as mps,
        ):
            # landmarks Qt~ = qsum/64 (bias already in), block-diag per pair
            blkq = []
            for p in range(NPAIR):
                bq_t = pers.tile([P, P], bf16, tag=f"blkq{p}")
                nc.vector.tensor_copy(bq_t[0:64, 64:128], zsrc[0:64, 0:64])
                nc.vector.tensor_copy(bq_t[64:128, 0:64], zsrc[0:64, 0:64])
                nc.vector.tensor_scalar_mul(bq_t[0:64, 0:64], qsum[p][0:64, :], 1.0 / 64)
                nc.vector.tensor_scalar_mul(bq_t[64:128, 64:128], qsum[p][64:128, :], 1.0 / 64)
                blkq.append(bq_t)

            # Gamma -> GD -> Newton-Schulz init (per-core scale, no collective)
            gstage = pers.tile([1, 2 * NPAIR], f32, tag="gstage")
            ones_row = pers.tile([1, P], f32, tag="ones_row")
            nc.vector.memset(ones_row[:], 1.0)
            vstate = []
            for p in range(NPAIR):
                psg = nsps.tile([P, 512], f32, tag="nsb")
                nc.tensor.matmul(psg[:, :P], blkq[p][:], blkq[p][:], start=True, stop=True)
                g = wk.tile([P, P], f32, tag="g")
                nc.scalar.activation(g[:], psg[:, :P], ACTF.Exp, scale=0.125)
                nc.vector.memset(g[0:64, 64:128], 0.0)
                nc.vector.memset(g[64:128, 0:64], 0.0)
                gs = wk.tile([P, 1], f32, tag="gs")
                nc.vector.reduce_sum(gs[:], g[:], axis=AX.X)
                gri = wk.tile([P, 1], f32, tag="gri")
                nc.vector.reciprocal(gri[:], gs[:])
                gd = nsp.tile([P, P], f32, tag=f"gd{p}", name=f"gd{p}")
                nc.vector.tensor_scalar_mul(gd[:], g[:], gri[:])

                pskt = nsps.tile([P, 512], f32, tag="nsb")
                nc.tensor.matmul(pskt[:, :P], gd[:], ident32[:], is_transpose=True)
                ktpad = nsp.tile([P, 256], f32r, tag=f"kt{p}")
                nc.vector.tensor_copy(ktpad[:, P:], zsrc[:])
                csum = wk.tile([P, 1], f32, tag="csum")
                nc.vector.tensor_scalar(
                    ktpad[:, :P], pskt[:, :P], 1.0, None, ALU.mult, ALU.add, accum_out=csum[:]
                )
                # partition_all_reduce only works at base partition 0:
                # separate the two heads into columns, pad with -1e30
                csep = wk.tile([P, 2], f32, tag="csep")
                nc.vector.memset(csep[:], -1e30)
                nc.vector.tensor_copy(csep[0:64, 0:1], csum[0:64, :])
                nc.vector.tensor_copy(csep[64:128, 1:2], csum[64:128, :])
                cmax = wk.tile([P, 2], f32, tag="cmax")
                nc.gpsimd.partition_all_reduce(
                    cmax[:], csep[:], P, bass_isa.ReduceOp.max
                )
                nc.vector.tensor_copy(gstage[0:1, 2 * p:2 * p + 2], cmax[0:1, 0:2])
                vstate.append([ktpad, gd])

            gmax = pers.tile([1, 1], f32, tag="gmax")
            nc.vector.reduce_max(gmax[:], gstage[:], axis=AX.X)
            psb = nsps.tile([P, 512], f32, tag="nsb")
            nc.tensor.matmul(psb[:, 0:1], ones_row[:], gmax[:], start=True, stop=True)
            sv_g = pers.tile([P, 1], f32, tag="sv_g")
            nc.vector.reciprocal(sv_g[:], psb[:, 0:1])

            for p in range(NPAIR):
                ktpad, gd = vstate[p]
                # vv packs [V | V^T] side by side; right halves zeroed once
                vv0 = nsp.tile([P, 2, 256], f32r, tag=f"vv{p}", name=f"vv0_{p}")
                nc.vector.tensor_copy(vv0[:, 0, P:], zsrc[:])
                nc.vector.tensor_copy(vv0[:, 1, P:], zsrc[:])
                nc.vector.tensor_scalar_mul(vv0[:, 0, 0:P], ktpad[:, :P], sv_g[:])
                # V0^T = s*K directly (s constant within each head block)
                nc.vector.tensor_scalar_mul(vv0[:, 1, 0:P], gd[:], sv_g[:])
                # a1/a3/a5 right halves are written once and stay zero
                # (their tags are per-pair with bufs=1 -> stable buffers)
                for tg in ("a1", "a3", "a5"):
                    az = nsp.tile([P, 256], f32r, tag=f"{tg}{p}", name=f"{tg}z_{p}")
                    nc.vector.tensor_copy(az[:, P:], zsrc[:])
                vstate[p] = [ktpad, vv0]

            # --- Newton-Schulz as a software-pipelined sub-stage stream ---
            # Each (it, pair) step = 4 sub-stages; consecutive sub-stages of
            # one pair are emitted ~1 loop-iteration apart so PE never waits
            # on the DVE/Act/Pool ops between its matmuls.
            ns_state = [dict(v=vstate[p][1][:, 0], v_l=vstate[p][1][:, 0, 0:P],
                             vt=vstate[p][1][:, 1], vt_l=vstate[p][1][:, 1, 0:P],
                             bank=None, kvt=None, a1=None, a3=None, a5=None)
                        for p in range(NPAIR)]

            def ns_substage(step):
                it, rem = divmod(step, 4 * NPAIR)
                s, p = divmod(rem, NPAIR)
                stt = ns_state[p]
                ktpad = vstate[p][0]
                if s == 0:
                    bank = nsps.tile([P, 512], f32, tag="nsb", name=f"kv{p}_{it}")
                    nc.tensor.matmul(bank[:, 0:256], ktpad[:, :P], stt["v"],
                                     start=True, stop=False, skip_group_check=True)
                    nc.tensor.matmul(bank[:, 256:512], stt["v_l"], ktpad[:],
                                     start=False, stop=True, skip_group_check=True)
                    kvt = nsp.tile([P, P], f32r, tag=f"kvt{p}", name=f"kvt{p}_{it}")
                    nc.scalar.copy(kvt[:], bank[:, 256:384])
                    a1 = nsp.tile([P, 256], f32r, tag=f"a1{p}", name=f"a1{p}_{it}")
                    nc.vector.tensor_tensor(a1[:, :P], i7[:], bank[:, 0:128], ALU.subtract)
                    stt["kvt"], stt["a1"] = kvt, a1
                elif s == 1:
                    bank = nsps.tile([P, 512], f32, tag="nsb", name=f"a24_{p}_{it}")
                    nc.tensor.matmul(bank[:, 0:256], stt["kvt"][:], stt["a1"][:],
                                     start=True, stop=False, skip_group_check=True)
                    a3 = nsp.tile([P, 256], f32r, tag=f"a3{p}", name=f"a3{p}_{it}")
                    nc.vector.tensor_tensor(a3[:, :P], i15[:], bank[:, 0:128], ALU.subtract)
                    stt["bank"], stt["a3"] = bank, a3
                elif s == 2:
                    bank = stt["bank"]
                    nc.tensor.matmul(bank[:, 256:512], stt["kvt"][:], stt["a3"][:],
                                     start=False, stop=True, skip_group_check=True)
                    a5 = nsp.tile([P, 256], f32r, tag=f"a5{p}", name=f"a5{p}_{it}")
                    nc.vector.tensor_tensor(a5[:, :P], i13[:], bank[:, 256:384], ALU.subtract)
                    stt["a5"] = a5
                else:
                    bank = nsps.tile([P, 512], f32, tag="nsb", name=f"vv{p}_{it}")
                    a5 = stt["a5"]
                    if it < NS_ITERS - 1:
                        nc.tensor.matmul(bank[:, 0:256], stt["vt_l"], a5[:],
                                         start=True, stop=False, skip_group_check=True)
                        nc.tensor.matmul(bank[:, 256:512], a5[:, :P], stt["vt"],
                                         start=False, stop=True, skip_group_check=True)
                        # one strided op writes both V and V^T (x0.25);
                        # the vv buffer's right halves were zeroed at init
                        vvn = nsp.tile([P, 2, 256], f32r, tag=f"vv{p}", name=f"vvn{p}_{it}")
                        nc.scalar.mul(
                            vvn[:, :, 0:P],
                            bank[:].rearrange("p (g x) -> p g x", x=256)[:, :, 0:P],
                            0.25,
                        )
                        stt["v"], stt["v_l"] = vvn[:, 0], vvn[:, 0, 0:P]
                        stt["vt"], stt["vt_l"] = vvn[:, 1], vvn[:, 1, 0:P]
                    else:
                        # v unused after the last iteration (W needs only vt)
                        nc.tensor.matmul(bank[:, 256:512], a5[:, :P], stt["vt"],
                                         start=True, stop=True, skip_group_check=True)
                        vtn = nsp.tile([P, 256], f32r, tag=f"vt{p}", name=f"vtn{p}_{it}")
                        nc.scalar.mul(vtn[:, :P], bank[:, 256:384], 0.25)
                        stt["vt"], stt["vt_l"] = vtn[:], vtn[:, 0:P]

            NS_TOTAL = 4 * NPAIR * NS_ITERS  # 96 sub-stages
            ns_step = [0]

            def ns_advance(n):
                e = min(ns_step[0] + n, NS_TOTAL)
                for k in range(ns_step[0], e):
                    ns_substage(k)
                ns_step[0] = e

            # Merged ST + token loop: per 512-token group, compute
            # st = exp(blkQ^T @ QKVT / 8) (c partials via accum_out), then
            # immediately run the 4 token-chunks (transposes, r row-sums,
            # M accumulation). 12 NS sub-stages interleaved per group.
            onesblk_s = pers.tile([P, 2], f32, tag="onesblk_s")
            nc.vector.memset(onesblk_s[:], 0.0)
            nc.vector.memset(onesblk_s[0:64, 0:1], 1.0)
            nc.vector.memset(onesblk_s[64:128, 1:2], 1.0)
            onesblk = pers.tile([P, 2], bf16, tag="onesblk")
            nc.vector.tensor_copy(onesblk[:], onesblk_s[:])
            cparts = []
            for p in range(NPAIR):
                cparts.append(pers.tile([P, NCH8], f32, tag=f"cpart{p}", name=f"cpart{p}"))
            mbank = [mps.tile([P, 512], f32, tag=f"mb{q}", name=f"mb{q}") for q in range(2)]
            rv = mps.tile([P, 256], f32, tag="rv", name="rv")
            for c8 in range(NCH8):
                for p in range(NPAIR):
                    psst = trps.tile([P, 512], f32, tag="trp", name=f"psst{p}_{c8}")
                    nc.tensor.matmul(
                        psst[:], blkq[p][:], qkvt[:, p, c8 * 512:(c8 + 1) * 512],
                        start=True, stop=True,
                    )
                    nc.scalar.activation(
                        st[:, p, c8 * 512:(c8 + 1) * 512], psst[:], ACTF.Exp,
                        scale=0.125, accum_out=cparts[p][:, c8:c8 + 1],
                    )
                qnbs = []
                for sub in range(4):
                    c = 4 * c8 + sub
                    tsl = slice(c * P, (c + 1) * P)
                    psq = trps.tile([P, 512], bf16, tag="trp", name=f"psq_{c}")
                    for p in range(NPAIR):
                        nc.tensor.matmul(
                            psq[:, p * P:(p + 1) * P], qkvt[:, p, tsl], identb[:],
                            is_transpose=True, start=(p == 0), stop=(p == NPAIR - 1),
                            skip_group_check=True,
                        )
                    qnb = snp.tile([P, 512], bf16, tag="qnb", name=f"qnb_{c}")
                    nc.vector.tensor_copy(qnb[:], psq[:])
                    qnbs.append(qnb)
                    ns_advance(1)
                for sub in range(4):
                    c = 4 * c8 + sub
                    tsl = slice(c * P, (c + 1) * P)
                    qnb = qnbs[sub]
                    qn = [qnb[:, 0:256], qnb[:, 256:512]]
                    pss = trps.tile([P, 512], bf16, tag="trp", name=f"pss_{c}")
                    for p in range(NPAIR):
                        nc.tensor.matmul(
                            pss[:, p * P:(p + 1) * P], st[:, p, tsl], identb[:],
                            is_transpose=True, start=(p == 0), stop=(p == NPAIR - 1),
                            skip_group_check=True,
                        )
                    sn_all = snp.tile([P, 512], bf16, tag="sna", name=f"sna_{c}")
                    nc.vector.tensor_copy(sn_all[:], pss[:])
                    for p in range(NPAIR):
                        nc.tensor.matmul(
                            rv[:, c * 8 + 2 * p:c * 8 + 2 * p + 2],
                            st[:, p, tsl], onesblk[:],
                            start=True, stop=True, skip_group_check=True,
                        )
                    ns_advance(2)
                    for q in range(2):
                        for j in range(2):
                            p = 2 * q + j
                            nc.tensor.matmul(
                                mbank[q][:, j * 256:(j + 1) * 256],
                                sn_all[:, p * P:(p + 1) * P], qn[q],
                                start=(c == 0 and j == 0),
                                stop=(c == TCH - 1 and j == 1),
                                skip_group_check=True,
                            )
            ns_advance(NS_TOTAL)  # drain any remainder
            rvi = pers.tile([P, 256], f32, tag="rvi")
            nc.vector.reciprocal(rvi[:], rv[:])

            # W = V6 @ (diag(1/c) M)
            wpads = []
            for p in range(NPAIR):
                q, j = divmod(p, 2)
                cs = wk.tile([P, 1], f32, tag="cs")
                nc.vector.reduce_sum(cs[:], cparts[p][:], axis=AX.X)
                cinv = wk.tile([P, 1], f32, tag="cinv")
                nc.vector.reciprocal(cinv[:], cs[:])
                dvp = wk.tile([P, 256], f32r, tag="dvp")
                nc.vector.tensor_copy(dvp[:, P:], zsrc[:])
                nc.vector.tensor_scalar_mul(
                    dvp[:, :P], mbank[q][:, j * 384:j * 384 + P], cinv[:]
                )
                # zero cross-head blocks (garbage from the paired-rhs M matmul)
                nc.vector.tensor_copy(dvp[0:64, 64:128], zsrc[0:64, 0:64])
                nc.vector.tensor_copy(dvp[64:128, 0:64], zsrc[0:64, 0:64])
                psw = nsps.tile([P, 512], f32, tag="nsb")
                nc.tensor.matmul(psw[:, 0:256], ns_state[p]["vt_l"], dvp[:], start=True, stop=True,
                                 skip_group_check=True)
                wpad = pers.tile([P, 256], bf16, tag=f"wpad{p}")
                nc.scalar.copy(wpad[:], psw[:, 0:256])
                wpads.append(wpad)

            # final: out = diag(1/r) S W  (2 pairs packed per psum bank);
            # outputs staged 4 chunks at a time for batched DMA
            ore = out_d.rearrange("(g c p) f -> g p c f", c=4, p=P)
            for c in range(TCH):
                tsl = slice(c * P, (c + 1) * P)
                if c % 4 == 0:
                    obuf = obp.tile([P, 4, 512], f32, tag="obuf", name=f"ob{c}")
                for q in range(2):
                    # alternate psum pools for a deeper final-stage pipeline
                    pool, tg = (trps, "trp") if (2 * c + q) % 5 < 3 else (nsps, "nsb")
                    pso = pool.tile([P, 512], f32, tag=tg, name=f"pso{q}_{c}")
                    for j in range(2):
                        p = 2 * q + j
                        nc.tensor.matmul(
                            pso[:, j * 256:j * 256 + 256], st[:, p, tsl], wpads[p][:],
                            start=(j == 0), stop=(j == 1), skip_group_check=True,
                        )
                    eng = nc.vector
                    eng.tensor_tensor(
                        obuf[:, c % 4, q * 256:(q + 1) * 256]
                            .rearrange("p (b h d) -> p b h d", h=2, d=64),
                        pso[:].rearrange("p (b n) -> p b n", n=256)[:, :, 0:128]
                            .rearrange("p b (h d) -> p b h d", d=64),
                        rvi[:, c * 8 + 4 * q:c * 8 + 4 * q + 4]
                            .rearrange("p (b h one) -> p b h one", h=2, one=1)
                            .to_broadcast([P, 2, 2, 64]),
                        ALU.mult,
                    )
                if c % 4 == 3:
                    dmae = nc.sync if c % 8 == 3 else nc.scalar
                    dmae.dma_start(ore[c // 4], obuf[:])

    nc.compile()
    return nc


def _get_nc():
    if "nc" not in _CACHE:
        _CACHE["nc"] = _build()
    return _CACHE["nc"]


def kernel(X, Wq, bq):
    from concourse.bass_utils import run_bass_kernel_spmd

    nc = _get_nc()
    B, E, n = X.shape
    H = Wq.shape[0]
    in_maps = []
    for core in range(8):
        b = core // 2
        h0 = 8 * (core % 2)
        wq_c = Wq[h0:h0 + 8]                      # [8, 64, 1024]
        wqt_c = np.ascontiguousarray(wq_c.transpose(2, 0, 1).reshape(E, 512))
        bias_c = np.ascontiguousarray(bq[h0:h0 + 8].reshape(512))
        in_maps.append({
            "X": np.ascontiguousarray(X[b]),
            "WqT": wqt_c,
            "bias": bias_c,
        })
    res = run_bass_kernel_spmd(nc, in_maps, core_ids=list(range(8)))
    out = np.empty((B, H, n, 64), dtype=np.float32)
    for core in range(8):
        b = core // 2
        h0 = 8 * (core % 2)
        oc = res.results[core]["out"].reshape(n, 8, 64)
        out[b, h0:h0 + 8] = oc.transpose(1, 0, 2)
    return out


# revision 30
# speedup vs baseline: 1.5389x; 1.0121x over previous
"""CoNystromAttention Trainium2 kernel.

Shard: 8 cores = 4 batches x 2 head-groups (8 heads each). Per core:
one batch b, 8 heads organized as 4 "pairs" (2 heads = 128 partitions).

Math (reference, with Q=K=V=QKV):
  QKV = X[b].T @ Wq[h].T + bq[h]                       [n=4096, d=64]
  Qt  = window-mean(QKV, 64)                           [m=64, d]
  S   = exp(QKV @ Qt.T / 8)     (Beta; Delta = S.T)    [n, m]
  G   = exp(Qt @ Qt.T / 8)                             [m, m]
  GD  = G / rowsum(G);  V6 = newton_schulz(GD, 6)      (pinv)
  out = diag(1/r) S V6 diag(1/c) S.T QKV,  r=rowsum(S), c=colsum(S)

Schedule: engines are in-order, so Newton-Schulz (a long serial
dependency chain) is software-pipelined as sub-stages interleaved into
the st/token loops where PE has idle slots. Bias is folded into the
projection as a rank-1 matmul; r=rowsum(S) via tiny ones-matmuls.
NS init scale uses the per-core max (NS converged by iter 6 for any
scale in range; verified rel-err 8.6e-4) so no collective is needed.
"""

import numpy as np

P = 128
N_TOK = 4096
EMBED = 1024
NPAIR = 4            # head-pairs per core (8 heads)
ECH = EMBED // P     # 8 contraction chunks
XCH = 512            # projection chunk (tokens)
NCHP = N_TOK // XCH  # 8 projection chunks
NCH8 = N_TOK // 512  # 8 ST chunks of 512
TCH = N_TOK // P     # 32 token chunks of 128
NS_ITERS = 6

_CACHE = {}


def _build():
    import concourse.mybir as mybir
    from concourse import bacc, bass_isa
    from concourse.tile import TileContext
    from concourse.masks import make_identity

    f32 = mybir.dt.float32
    f32r = mybir.dt.float32r
    bf16 = mybir.dt.bfloat16
    ALU = mybir.AluOpType
    ACTF = mybir.ActivationFunctionType
    AX = mybir.AxisListType

    def r(ap):
        return ap.bitcast(f32r)

    nc = bacc.Bacc("TRN2", target_bir_lowering=False, debug=False)
    X = nc.dram_tensor("X", [EMBED, N_TOK], f32, kind="ExternalInput")
    WqT = nc.dram_tensor("WqT", [EMBED, 512], f32, kind="ExternalInput")
    bias = nc.dram_tensor("bias", [512], f32, kind="ExternalInput")
    out_d = nc.dram_tensor("out", [N_TOK, 512], f32, kind="ExternalOutput")

    with TileContext(nc) as tc, (
        tc.tile_pool(name="big", bufs=1)
    ) as big, tc.tile_pool(name="persist", bufs=1) as pers, tc.tile_pool(
        name="nsv", bufs=1
    ) as nsp:
        # ---------------- persistent tiles ----------------
        ident32 = pers.tile([P, P], f32, tag="ident32")
        make_identity(nc, ident32[:])
        i7 = pers.tile([P, P], f32, tag="i7")
        i15 = pers.tile([P, P], f32, tag="i15")
        i13 = pers.tile([P, P], f32, tag="i13")
        for t, v in ((i7, 7.0), (i15, 15.0), (i13, 13.0)):
            nc.vector.tensor_scalar_mul(t[:], ident32[:], v)
        # bias on partition 0 as [1, 512] for the rank-1 bias matmul
        bias_st = pers.tile([1, 512], f32, tag="bias_st")
        nc.sync.dma_start(bias_st[:], bias.rearrange("(o f) -> o f", o=1))
        bias_row = pers.tile([1, 512], f32r, tag="bias_row")
        nc.vector.tensor_copy(bias_row[:], bias_st[:])
        ones512s = pers.tile([1, 512], f32, tag="ones512s")
        nc.vector.memset(ones512s[:], 1.0)
        ones512 = pers.tile([1, 512], f32r, tag="ones512")
        nc.vector.tensor_copy(ones512[:], ones512s[:])
        identb = pers.tile([P, P], bf16, tag="identb")
        nc.vector.tensor_copy(identb[:], ident32[:])
        zsrc = pers.tile([P, P], f32, tag="zsrc")
        nc.vector.memset(zsrc[:], 0.0)
        qsum = [pers.tile([P, 64], f32, tag=f"qsum{p}", name=f"qsum{p}") for p in range(NPAIR)]
        qkvt = big.tile([P, NPAIR, N_TOK], bf16, tag="qkvt")
        st = big.tile([P, NPAIR, N_TOK], bf16, tag="st")

        # ---------------- phase 1: projection ----------------
        with (
            tc.tile_pool(name="wq", bufs=1) as wqp,
            tc.tile_pool(name="x", bufs=4) as xpool,
            tc.tile_pool(name="pp", bufs=8, space="PSUM") as pp,
        ):
            wqts = wqp.tile([P, ECH, 512], f32, tag="wqts")
            wqtr = wqp.tile([P, ECH, 512], f32r, tag="wqtr")
            wre = WqT.rearrange("(eo p) hd -> p eo hd", p=P)
            # first weight block + first X quarters lead; late weight blocks
            # trail behind chunk 0's inputs so the first matmuls start early
            nc.sync.dma_start(wqts[:, 0:2], wre[:, 0:2])
            nc.scalar.dma_start(wqts[:, 2:5], wre[:, 2:5])
            nc.scalar.copy(wqtr[:, 0:2], wqts[:, 0:2])
            nc.gpsimd.tensor_copy(wqtr[:, 2:5], wqts[:, 2:5])

            xre = X.rearrange("(eo p) n -> p eo n", p=P)
            for c in range(NCHP):
                xts = []
                for quarter in range(4):
                    xt = xpool.tile([P, ECH // 4, XCH], f32, tag="xt")
                    dmae = nc.sync if quarter % 2 == 0 else nc.scalar
                    dmae.dma_start(
                        xt[:],
                        xre[:, quarter * 2:(quarter + 1) * 2, c * XCH:(c + 1) * XCH],
                    )
                    xr = xpool.tile([P, ECH // 4, XCH], f32r, tag="xr")
                    if c == 0:
                        eng = (nc.vector, nc.gpsimd, nc.scalar, nc.gpsimd)[quarter]
                        eng.tensor_copy(xr[:], xt[:]) if eng is not nc.scalar \
                            else nc.scalar.copy(xr[:], xt[:])
                    elif quarter % 2 == 0:
                        nc.scalar.copy(xr[:], xt[:])
                    else:
                        nc.gpsimd.tensor_copy(xr[:], xt[:])
                    xts.append(xr)
                if c == 0:
                    nc.sync.dma_start(wqts[:, 5:8], wre[:, 5:8])
                    nc.scalar.copy(wqtr[:, 5:8], wqts[:, 5:8])
                # e-outer, p-inner: each input quarter is consumed after 2
                # e-steps, freeing its buffer for chunk c+1's DMA early
                pss_ = [pp.tile([P, XCH], f32, tag="proj", name=f"proj{p}_{c}")
                        for p in range(NPAIR)]
                for e in range(ECH):
                    for p in range(NPAIR):
                        nc.tensor.matmul(
                            pss_[p][:],
                            wqtr[:, e, p * P:(p + 1) * P],
                            xts[e // 2][:, e % 2, :],
                            start=(e == 0),
                            stop=False,
                        )
                for p in range(NPAIR):
                    ps = pss_[p]
                    # + bias (rank-1: bias_row^T @ ones)
                    nc.tensor.matmul(
                        ps[:],
                        bias_row[:, p * P:(p + 1) * P],
                        ones512[:, :XCH],
                        start=False,
                        stop=True,
                        skip_group_check=True,
                    )
                    if p % 2 == 0:
                        nc.scalar.copy(qkvt[:, p, c * XCH:(c + 1) * XCH], ps[:])
                    else:
                        nc.vector.tensor_copy(qkvt[:, p, c * XCH:(c + 1) * XCH], ps[:])
                    # landmark partial sums (bias included): 8 windows/chunk
                    nc.vector.reduce_sum(
                        qsum[p][:, c * 8:(c + 1) * 8],
                        ps[:].rearrange("p (w t) -> p w t", t=64),
                        axis=AX.X,
                    )

        # ---------------- phase 2 ----------------
        with (
            tc.tile_pool(name="wk", bufs=4) as wk,
            tc.tile_pool(name="sn", bufs=4) as snp,
            tc.tile_pool(name="ob", bufs=3) as obp,
            tc.tile_pool(name="nsps", bufs=2, space="PSUM") as nsps,
            tc.tile_pool(name="trps", bufs=3, space="PSUM") as trps,
            tc.tile_pool(name="mps", bufs=1, space="PSUM") as mps,
        ):
            # landmarks Qt~ = qsum/64 (bias already in), block-diag per pair
            blkq = []
            for p in range(NPAIR):
                bq_t = pers.tile([P, P], bf16, tag=f"blkq{p}")
                nc.vector.tensor_copy(bq_t[0:64, 64:128], zsrc[0:64, 0:64])
                nc.vector.tensor_copy(bq_t[64:128, 0:64], zsrc[0:64, 0:64])
                nc.vector.tensor_scalar_mul(bq_t[0:64, 0:64], qsum[p][0:64, :], 1.0 / 64)
                nc.vector.tensor_scalar_mul(bq_t[64:128, 64:128], qsum[p][64:128, :], 1.0 / 64)
                blkq.append(bq_t)

            # Gamma -> GD -> Newton-Schulz init (per-core scale, no collective)
            gstage = pers.tile([1, 2 * NPAIR], f32, tag="gstage")
            ones_row = pers.tile([1, P], f32, tag="ones_row")
            nc.vector.memset(ones_row[:], 1.0)
            vstate = []
            for p in range(NPAIR):
                psg = nsps.tile([P, 512], f32, tag="nsb")
                nc.tensor.matmul(psg[:, :P], blkq[p][:], blkq[p][:], start=True, stop=True)
                g = wk.tile([P, P], f32, tag="g")
                nc.scalar.activation(g[:], psg[:, :P], ACTF.Exp, scale=0.125)
                nc.vector.memset(g[0:64, 64:128], 0.0)
                nc.vector.memset(g[64:128, 0:64], 0.0)
                gs = wk.tile([P, 1], f32, tag="gs")
                nc.vector.reduce_sum(gs[:], g[:], axis=AX.X)
                gri = wk.tile([P, 1], f32, tag="gri")
                nc.vector.reciprocal(gri[:], gs[:])
                gd = nsp.tile([P, P], f32, tag=f"gd{p}", name=f"gd{p}")
                nc.vector.tensor_scalar_mul(gd[:], g[:], gri[:])

                pskt = nsps.tile([P, 512], f32, tag="nsb")
                nc.tensor.matmul(pskt[:, :P], gd[:], ident32[:], is_transpose=True)
                ktpad = nsp.tile([P, 256], f32r, tag=f"kt{p}")
                nc.vector.tensor_copy(ktpad[:, P:], zsrc[:])
                csum = wk.tile([P, 1], f32, tag="csum")
                nc.vector.tensor_scalar(
                    ktpad[:, :P], pskt[:, :P], 1.0, None, ALU.mult, ALU.add, accum_out=csum[:]
                )
                # partition_all_reduce only works at base partition 0:
                # separate the two heads into columns, pad with -1e30
                csep = wk.tile([P, 2], f32, tag="csep")
                nc.vector.memset(csep[:], -1e30)
                nc.vector.tensor_copy(csep[0:64, 0:1], csum[0:64, :])
                nc.vector.tensor_copy(csep[64:128, 1:2], csum[64:128, :])
                cmax = wk.tile([P, 2], f32, tag="cmax")
                nc.gpsimd.partition_all_reduce(
                    cmax[:], csep[:], P, bass_isa.ReduceOp.max
                )
                nc.vector.tensor_copy(gstage[0:1, 2 * p:2 * p + 2], cmax[0:1, 0:2])
                vstate.append([ktpad, gd])

            gmax = pers.tile([1, 1], f32, tag="gmax")
            nc.vector.reduce_max(gmax[:], gstage[:], axis=AX.X)
            psb = nsps.tile([P, 512], f32, tag="nsb")
            nc.tensor.matmul(psb[:, 0:1], ones_row[:], gmax[:], start=True, stop=True)
            sv_g = pers.tile([P, 1], f32, tag="sv_g")
            nc.vector.reciprocal(sv_g[:], psb[:, 0:1])

            for p in range(NPAIR):
                ktpad, gd = vstate[p]
                # vv packs [V | V^T] side by side; right halves zeroed once
                vv0 = nsp.tile([P, 2, 256], f32r, tag=f"vv{p}", name=f"vv0_{p}")
                nc.vector.tensor_copy(vv0[:, 0, P:], zsrc[:])
                nc.vector.tensor_copy(vv0[:, 1, P:], zsrc[:])
                nc.vector.tensor_scalar_mul(vv0[:, 0, 0:P], ktpad[:, :P], sv_g[:])
                # V0^T = s*K directly (s constant within each head block)
                nc.vector.tensor_scalar_mul(vv0[:, 1, 0:P], gd[:], sv_g[:])
                # a1/a3/a5 right halves are written once and stay zero
                # (their tags are per-pair with bufs=1 -> stable buffers)
                for tg in ("a1", "a3", "a5"):
                    az = nsp.tile([P, 256], f32r, tag=f"{tg}{p}", name=f"{tg}z_{p}")
                    nc.vector.tensor_copy(az[:, P:], zsrc[:])
                vstate[p] = [ktpad, vv0]

            # --- Newton-Schulz as a software-pipelined sub-stage stream ---
            # Each (it, pair) step = 4 sub-stages; consecutive sub-stages of
            # one pair are emitted ~1 loop-iteration apart so PE never waits
            # on the DVE/Act/Pool ops between its matmuls.
            ns_state = [dict(v=vstate[p][1][:, 0], v_l=vstate[p][1][:, 0, 0:P],
                             vt=vstate[p][1][:, 1], vt_l=vstate[p][1][:, 1, 0:P],
                             bank=None, kvt=None, a1=None, a3=None, a5=None)
                        for p in range(NPAIR)]

            def ns_substage(step):
                it, rem = divmod(step, 4 * NPAIR)
                s, p = divmod(rem, NPAIR)
                stt = ns_state[p]
                ktpad = vstate[p][0]
                if s == 0:
                    bank = nsps.tile([P, 512], f32, tag="nsb", name=f"kv{p}_{it}")
                    nc.tensor.matmul(bank[:, 0:256], ktpad[:, :P], stt["v"],
                                     start=True, stop=False, skip_group_check=True)
                    nc.tensor.matmul(bank[:, 256:512], stt["v_l"], ktpad[:],
                                     start=False, stop=True, skip_group_check=True)
                    kvt = nsp.tile([P, P], f32r, tag=f"kvt{p}", name=f"kvt{p}_{it}")
                    nc.scalar.copy(kvt[:], bank[:, 256:384])
                    a1 = nsp.tile([P, 256], f32r, tag=f"a1{p}", name=f"a1{p}_{it}")
                    nc.vector.tensor_tensor(a1[:, :P], i7[:], bank[:, 0:128], ALU.subtract)
                    stt["kvt"], stt["a1"] = kvt, a1
                elif s == 1:
                    bank = nsps.tile([P, 512], f32, tag="nsb", name=f"a24_{p}_{it}")
                    nc.tensor.matmul(bank[:, 0:256], stt["kvt"][:], stt["a1"][:],
                                     start=True, stop=False, skip_group_check=True)
                    a3 = nsp.tile([P, 256], f32r, tag=f"a3{p}", name=f"a3{p}_{it}")
                    nc.vector.tensor_tensor(a3[:, :P], i15[:], bank[:, 0:128], ALU.subtract)
                    stt["bank"], stt["a3"] = bank, a3
                elif s == 2:
                    bank = stt["bank"]
                    nc.tensor.matmul(bank[:, 256:512], stt["kvt"][:], stt["a3"][:],
                                     start=False, stop=True, skip_group_check=True)
                    a5 = nsp.tile([P, 256], f32r, tag=f"a5{p}", name=f"a5{p}_{it}")
                    nc.vector.tensor_tensor(a5[:, :P], i13[:], bank[:, 256:384], ALU.subtract)
                    stt["a5"] = a5
                else:
                    bank = nsps.tile([P, 512], f32, tag="nsb", name=f"vv{p}_{it}")
                    a5 = stt["a5"]
                    if it < NS_ITERS - 1:
                        nc.tensor.matmul(bank[:, 0:256], stt["vt_l"], a5[:],
                                         start=True, stop=False, skip_group_check=True)
                        nc.tensor.matmul(bank[:, 256:512], a5[:, :P], stt["vt"],
                                         start=False, stop=True, skip_group_check=True)
                        # one strided op writes both V and V^T (x0.25);
                        # the vv buffer's right halves were zeroed at init
                        vvn = nsp.tile([P, 2, 256], f32r, tag=f"vv{p}", name=f"vvn{p}_{it}")
                        nc.scalar.mul(
                            vvn[:, :, 0:P],
                            bank[:].rearrange("p (g x) -> p g x", x=256)[:, :, 0:P],
                            0.25,
                        )
                        stt["v"], stt["v_l"] = vvn[:, 0], vvn[:, 0, 0:P]
                        stt["vt"], stt["vt_l"] = vvn[:, 1], vvn[:, 1, 0:P]
                    else:
                        # v unused after the last iteration (W needs only vt)
                        nc.tensor.matmul(bank[:, 256:512], a5[:, :P], stt["vt"],
                                         start=True, stop=True, skip_group_check=True)
                        vtn = nsp.tile([P, 256], f32r, tag=f"vt{p}", name=f"vtn{p}_{it}")
                        nc.scalar.mul(vtn[:, :P], bank[:, 256:384], 0.25)
                        stt["vt"], stt["vt_l"] = vtn[:], vtn[:, 0:P]

            NS_TOTAL = 4 * NPAIR * NS_ITERS  # 96 sub-stages
            ns_step = [0]

            def ns_advance(n):
                e = min(ns_step[0] + n, NS_TOTAL)
                for k in range(ns_step[0], e):
                    ns_substage(k)
                ns_step[0] = e

            # Merged ST + token loop: per 512-token group, compute
            # st = exp(blkQ^T @ QKVT / 8) (c partials via accum_out), then
            # immediately run the 4 token-chunks (transposes, r row-sums,
            # M accumulation). 12 NS sub-stages interleaved per group.
            onesblk_s = pers.tile([P, 2], f32, tag="onesblk_s")
            nc.vector.memset(onesblk_s[:], 0.0)
            nc.vector.memset(onesblk_s[0:64, 0:1], 1.0)
            nc.vector.memset(onesblk_s[64:128, 1:2], 1.0)
            onesblk = pers.tile([P, 2], bf16, tag="onesblk")
            nc.vector.tensor_copy(onesblk[:], onesblk_s[:])
            cparts = []
            for p in range(NPAIR):
                cparts.append(pers.tile([P, NCH8], f32, tag=f"cpart{p}", name=f"cpart{p}"))
            mbank = [mps.tile([P, 512], f32, tag=f"mb{q}", name=f"mb{q}") for q in range(2)]
            rv = mps.tile([P, 256], f32, tag="rv", name="rv")
            for c8 in range(NCH8):
                for p in range(NPAIR):
                    psst = trps.tile([P, 512], f32, tag="trp", name=f"psst{p}_{c8}")
                    nc.tensor.matmul(
                        psst[:], blkq[p][:], qkvt[:, p, c8 * 512:(c8 + 1) * 512],
                        start=True, stop=True,
                    )
                    nc.scalar.activation(
                        st[:, p, c8 * 512:(c8 + 1) * 512], psst[:], ACTF.Exp,
                        scale=0.125, accum_out=cparts[p][:, c8:c8 + 1],
                    )
                qnbs = []
                for sub in range(4):
                    c = 4 * c8 + sub
                    tsl = slice(c * P, (c + 1) * P)
                    psq = trps.tile([P, 512], bf16, tag="trp", name=f"psq_{c}")
                    for p in range(NPAIR):
                        nc.tensor.matmul(
                            psq[:, p * P:(p + 1) * P], qkvt[:, p, tsl], identb[:],
                            is_transpose=True, start=(p == 0), stop=(p == NPAIR - 1),
                            skip_group_check=True,
                        )
                    qnb = snp.tile([P, 512], bf16, tag="qnb", name=f"qnb_{c}")
                    if sub % 2 == 0:
                        nc.scalar.copy(qnb[:], psq[:])
                    else:
                        nc.vector.tensor_copy(qnb[:], psq[:])
                    qnbs.append(qnb)
                    ns_advance(1)
                for sub in range(4):
                    c = 4 * c8 + sub
                    tsl = slice(c * P, (c + 1) * P)
                    qnb = qnbs[sub]
                    qn = [qnb[:, 0:256], qnb[:, 256:512]]
                    pss = trps.tile([P, 512], bf16, tag="trp", name=f"pss_{c}")
                    for p in range(NPAIR):
                        nc.tensor.matmul(
                            pss[:, p * P:(p + 1) * P], st[:, p, tsl], identb[:],
                            is_transpose=True, start=(p == 0), stop=(p == NPAIR - 1),
                            skip_group_check=True,
                        )
                    sn_all = snp.tile([P, 512], bf16, tag="sna", name=f"sna_{c}")
                    nc.vector.tensor_copy(sn_all[:], pss[:])
                    for p in range(NPAIR):
                        nc.tensor.matmul(
                            rv[:, c * 8 + 2 * p:c * 8 + 2 * p + 2],
                            st[:, p, tsl], onesblk[:],
                            start=True, stop=True, skip_group_check=True,
                        )
                    ns_advance(2)
                    for q in range(2):
                        for j in range(2):
                            p = 2 * q + j
                            nc.tensor.matmul(
                                mbank[q][:, j * 256:(j + 1) * 256],
                                sn_all[:, p * P:(p + 1) * P], qn[q],
                                start=(c == 0 and j == 0),
                                stop=(c == TCH - 1 and j == 1),
                                skip_group_check=True,
                            )
            ns_advance(NS_TOTAL)  # drain any remainder
            rvi = pers.tile([P, 256], f32, tag="rvi")
            nc.vector.reciprocal(rvi[:], rv[:])

            # W = V6 @ (diag(1/c) M)
            wpads = []
            for p in range(NPAIR):
                q, j = divmod(p, 2)
                cs = wk.tile([P, 1], f32, tag="cs")
                nc.vector.reduce_sum(cs[:], cparts[p][:], axis=AX.X)
                cinv = wk.tile([P, 1], f32, tag="cinv")
                nc.vector.reciprocal(cinv[:], cs[:])
                dvp = wk.tile([P, 256], f32r, tag="dvp")
                nc.vector.tensor_copy(dvp[:, P:], zsrc[:])
                nc.vector.tensor_scalar_mul(
                    dvp[:, :P], mbank[q][:, j * 384:j * 384 + P], cinv[:]
                )
                # zero cross-head blocks (garbage from the paired-rhs M matmul)
                nc.vector.tensor_copy(dvp[0:64, 64:128], zsrc[0:64, 0:64])
                nc.vector.tensor_copy(dvp[64:128, 0:64], zsrc[0:64, 0:64])
                psw = nsps.tile([P, 512], f32, tag="nsb")
                nc.tensor.matmul(psw[:, 0:256], ns_state[p]["vt_l"], dvp[:], start=True, stop=True,
                                 skip_group_check=True)
                wpad = pers.tile([P, 256], bf16, tag=f"wpad{p}")
                nc.scalar.copy(wpad[:], psw[:, 0:256])
                wpads.append(wpad)

            # final: out = diag(1/r) S W  (2 pairs packed per psum bank);
            # outputs staged 4 chunks at a time for batched DMA
            ore = out_d.rearrange("(g c p) f -> g p c f", c=4, p=P)
            for c in range(TCH):
                tsl = slice(c * P, (c + 1) * P)
                if c % 4 == 0:
                    obuf = obp.tile([P, 4, 512], f32, tag="obuf", name=f"ob{c}")
                for q in range(2):
                    # alternate psum pools for a deeper final-stage pipeline
                    pool, tg = (trps, "trp") if (2 * c + q) % 5 < 3 else (nsps, "nsb")
                    pso = pool.tile([P, 512], f32, tag=tg, name=f"pso{q}_{c}")
                    for j in range(2):
                        p = 2 * q + j
                        nc.tensor.matmul(
                            pso[:, j * 256:j * 256 + 256], st[:, p, tsl], wpads[p][:],
                            start=(j == 0), stop=(j == 1), skip_group_check=True,
                        )
                    eng = nc.vector
                    eng.tensor_tensor(
                        obuf[:, c % 4, q * 256:(q + 1) * 256]
                            .rearrange("p (b h d) -> p b h d", h=2, d=64),
                        pso[:].rearrange("p (b n) -> p b n", n=256)[:, :, 0:128]
                            .rearrange("p b (h d) -> p b h d", d=64),
                        rvi[:, c * 8 + 4 * q:c * 8 + 4 * q + 4]
                            .rearrange("p (b h one) -> p b h one", h=2, one=1)
                            .to_broadcast([P, 2, 2, 64]),
                        ALU.mult,
                    )
                if c % 4 == 3:
                    dmae = (nc.sync, nc.scalar, nc.gpsimd)[(c // 4) % 3]
                    dmae.dma_start(ore[c // 4], obuf[:])

    nc.compile()
    return nc


def _get_nc():
    if "nc" not in _CACHE:
        _CACHE["nc"] = _build()
    return _CACHE["nc"]


def kernel(X, Wq, bq):
    from concourse.bass_utils import run_bass_kernel_spmd

    nc = _get_nc()
    B, E, n = X.shape
    H = Wq.shape[0]
    in_maps = []
    for core in range(8):
        b = core // 2
        h0 = 8 * (core % 2)
        wq_c = Wq[h0:h0 + 8]                      # [8, 64, 1024]
        wqt_c = np.ascontiguousarray(wq_c.transpose(2, 0, 1).reshape(E, 512))
        bias_c = np.ascontiguousarray(bq[h0:h0 + 8].reshape(512))
        in_maps.append({
            "X": np.ascontiguousarray(X[b]),
            "WqT": wqt_c,
            "bias": bias_c,
        })
    res = run_bass_kernel_spmd(nc, in_maps, core_ids=list(range(8)))
    out = np.empty((B, H, n, 64), dtype=np.float32)
    for core in range(8):
        b = core // 2
        h0 = 8 * (core % 2)
        oc = res.results[core]["out"].reshape(n, 8, 64)
        out[b, h0:h0 + 8] = oc.transpose(1, 0, 2)
    return out
